# revision 2
# baseline (speedup 1.0000x reference)
"""
Trainium2 Bass kernel for nn_EventMotionModel (dense transformer block) — v2.

Same math/layout strategy as v1 (feature-major acts, batch-parallel over 8
cores, bf16 matmuls, fp32 psum) with engine-load rebalancing on top:

  * Host-side zero-col-mean folding of ew1/eb1, ew2/eb2, rw/rb, wo.
    LayerNorm is shift-invariant, so subtracting each weight column-mean
    makes the encoder pre-LN activations exactly zero-mean: the mean
    matmul passes, the mean/m2 stat ops, and the mean-subtract in the LN
    applies all disappear for LN1/LN2; the coupling LN reuses mean(h)
    from the query-LN stats (res and o@wo are zero-mean by construction).
  * All elementwise chains run in bf16 (2x/4x DVE modes) with per-LN
    fp32->bf16 casts of the stats; squares for sum-of-squares run on DVE
    as y*y instead of ACT Square.
  * LN applies fuse gamma/beta(+relu) into one ACT op via per-partition
    scale AND bias APs: h = Relu(t*gamma + beta), t = y*rstd.
  * Attention scores: one 128-wide matmul per (block, head) instead of
    four 32-wide ones; softmax normalize/mask muls in bf16; attention
    output evicted with one FD=512 strided ACT op per psum half.
"""

import numpy as np

import concourse.bass as bass
import concourse.tile as tile
from concourse import mybir
from concourse.bass import ds
from concourse.bass_utils import run_bass_kernel_spmd

# ---------------------------------------------------------------- constants
H = 1024
NH = 8
DH = 128
IN = 512
CD = 256
OUT = 512
FH = 128
B, T = 2048, 32
D = IN + CD  # 768

NCORES = 8
B_LOC = B // NCORES          # 256
NTOK = B_LOC * T             # 8192 tokens per core
TT = 512                     # tokens per tile
NBLK = TT // 128             # 128-token blocks per tile (= 4)

FP32 = mybir.dt.float32
BF16 = mybir.dt.bfloat16
AF = mybir.ActivationFunctionType
ALU = mybir.AluOpType

KO_X = D // 128              # 6 feature chunks of x
KO_H = H // 128              # 8 feature chunks of hidden

# packed per-feature vectors: name -> n_cols (=len/128) in the "vecs" input
VEC_SPECS = [
    ("eb1", 8), ("eg1", 8), ("ebt1", 8),
    ("eb2", 8), ("eg2", 8), ("ebt2", 8),
    ("rb", 8),
    ("lnq_g", 8), ("lnq_b", 8),
    ("cn_g", 8), ("cn_b", 8),
    ("hb1", 8),
    ("fb1", 1), ("fb2", 16),
]
VEC_OFF = {}
_off = 0
for _name, _n in VEC_SPECS:
    VEC_OFF[_name] = _off
    _off += _n
VEC_COLS = _off


# ---------------------------------------------------------------- program
def build_program(ntok=NTOK, tt=TT, reps=1, fixed_addr=False):
    import concourse.tile_sem_assignment as _tsa
    _tsa.NUM_HWDGE_SEMS = 4
    nblk = tt // 128
    nc = bass.Bass()

    # DRAM parameters ------------------------------------------------------
    x_fm = nc.declare_dram_parameter("x_fm", [D, ntok], BF16, isOutput=False)
    vecs_d = nc.declare_dram_parameter("vecs", [128, VEC_COLS], FP32, isOutput=False)
    hb2bc_d = nc.declare_dram_parameter("hb2bc", [128, OUT], FP32, isOutput=False)
    bdt_d = nc.declare_dram_parameter("bdt", [128, NH * 128], BF16, isOutput=False)
    w_d = {}
    for name, k, m in [
        ("ew1", D, H), ("ew2", H, H), ("rw", D, H),
        ("fw1", CD, FH), ("fw2", FH, 2 * H),
        ("wq", H, H), ("wk", H, H), ("wv", H, H), ("wo", H, H),
        ("hw1", H, H), ("hw2", H, OUT),
    ]:
        w_d[name] = nc.declare_dram_parameter(name, [k, m], BF16, isOutput=False)
    out_d = nc.declare_dram_parameter("out_tm", [ntok, OUT], FP32, isOutput=True)

    from contextlib import ExitStack

    with tile.TileContext(nc) as tc, ExitStack() as st, \
            nc.allow_low_precision(
                reason="bf16 intermediates feed bf16 matmuls; stats keep "
                       "fp32 until the final per-token scalars"):
        singles = st.enter_context(tc.tile_pool(name="singles", bufs=1))
        acts = st.enter_context(tc.tile_pool(name="acts", bufs=1))
        wpool = st.enter_context(tc.tile_pool(name="wpool", bufs=6))
        wvpool = st.enter_context(tc.tile_pool(name="wvpool", bufs=2))
        rwpool = st.enter_context(tc.tile_pool(name="rwpool", bufs=4))
        tmps = st.enter_context(tc.tile_pool(name="tmps", bufs=2))
        stat = st.enter_context(tc.tile_pool(name="stat", bufs=2))
        outp = st.enter_context(tc.tile_pool(name="outp", bufs=2))
        attp = st.enter_context(tc.tile_pool(name="attp", bufs=2))
        psum = st.enter_context(tc.tile_pool(name="psum", bufs=3, space="PSUM"))
        psatt = st.enter_context(tc.tile_pool(name="psatt", bufs=2, space="PSUM"))
        psout = st.enter_context(tc.tile_pool(name="psout", bufs=1, space="PSUM"))

        # resident constants ----------------------------------------------
        vecs = singles.tile([128, VEC_COLS], FP32)
        nc.sync.dma_start(vecs, vecs_d[:, :])
        hb2bc = singles.tile([128, OUT], FP32)
        nc.sync.dma_start(hb2bc, hb2bc_d[:, :])
        bdt = singles.tile([128, NH * 128], BF16)
        nc.sync.dma_start(bdt, bdt_d[:, :])
        ones = singles.tile([128, 128], BF16)
        nc.vector.memset(ones, 1.0)
        eps_sb = singles.tile([128, 1], FP32)
        nc.vector.memset(eps_sb, 1e-5)

        def vec(name, c):
            return vecs[:, VEC_OFF[name] + c : VEC_OFF[name] + c + 1]

        # weight streaming: load a [128, ko_n, m_n] slab of W
        def load_w(name, ko0, ko_n, m0, m_n, pool=None, tag="w"):
            w3 = w_d[name].rearrange("(ko p) m -> p ko m", p=128)
            t = (pool or wpool).tile([128, ko_n, m_n], BF16, tag=tag, name=f"w_{name}_{ko0}_{m0}")
            nc.sync.dma_start(t, w3[:, ko0 : ko0 + ko_n, m0 : m0 + m_n])
            return t

        # dense feature-major layer: act_chunks (list of [128, tt] APs) @ W.
        # consumer(mi, ps) receives each output chunk's psum [128, tt].
        def preload(name, KO, M, pool=None, unique_tags=False):
            kh = (KO + 1) // 2
            out = {}
            for m0 in range(0, M, 512):
                m_n = min(512, M - m0)
                out[m0] = [(k0, load_w(name, k0, min(kh, KO - k0), m0, m_n,
                                       pool=pool,
                                       tag=(f"w_{name}_{m0}_{k0}"
                                            if unique_tags else "w")))
                           for k0 in range(0, KO, kh)]
            return out

        def fm_layer(name, KO, M, act_of, consumer, slabs_by_m0=None):
            kh = (KO + 1) // 2  # K-halves: 3+3 for 768, 4+4 for 1024
            for m0 in range(0, M, 512):
                m_n = min(512, M - m0)
                if slabs_by_m0 is not None:
                    slabs = slabs_by_m0[m0]
                else:
                    slabs = []
                    for k0 in range(0, KO, kh):
                        k_n = min(kh, KO - k0)
                        slabs.append((k0, load_w(name, k0, k_n, m0, m_n)))
                for mi in range(m_n // 128):
                    ps = psum.tile([128, tt], FP32, tag="mm")
                    for k0, slab in slabs:
                        k_n = slab.shape[1]
                        for kk in range(k_n):
                            ko = k0 + kk
                            nc.tensor.matmul(
                                ps,
                                lhsT=slab[:, kk, mi * 128 : (mi + 1) * 128],
                                rhs=act_of(ko),
                                start=(ko == 0),
                                stop=(ko == KO - 1),
                            )
                    consumer(m0 // 128 + mi, ps)

        # sum of squares over KO chunks of y -> psum [128, tt] (broadcast
        # over partitions).  Squares on DVE in bf16.
        def ssq_psum(y_of, KO, on_act=False):
            ps_ssq = psum.tile([128, tt], FP32, tag="mm")
            for c in range(KO):
                sq = tmps.tile([128, tt], BF16, tag="sq")
                if on_act:
                    nc.scalar.square(sq, y_of(c))
                else:
                    nc.vector.tensor_mul(sq, y_of(c), y_of(c))
                nc.tensor.matmul(ps_ssq, lhsT=ones, rhs=sq,
                                 start=(c == 0), stop=(c == KO - 1))
            return ps_ssq

        # rstd (bf16) for a zero-mean y: rstd = 1/sqrt(ssq/n + eps)
        def recip_cast(sd):
            rstd = stat.tile([128, tt], BF16, tag="rstd_bf")
            nc.vector.reciprocal(rstd, sd)
            return rstd

        def rstd_zeromean(y_of, KO):
            ps_ssq = ssq_psum(y_of, KO)
            sd = stat.tile([128, tt], FP32, tag="sd")
            nc.scalar.activation(sd, ps_ssq, AF.Sqrt, bias=eps_sb,
                                 scale=1.0 / float(KO * 128))
            return recip_cast(sd)

        # resident ew2 (2MB bf16): saves 2MB/tile of weight streaming
        ew2_res = preload("ew2", KO_H, H, pool=singles, unique_tags=True)

        # ---------------------------------------------------------- loop
        with tc.For_i(0, ntok * reps, tt) as it_raw:
            it = 0 if fixed_addr else it_raw
            x_sb = acts.tile([128, KO_X, tt], BF16, tag="slotA")  # x
            xv = x_fm.rearrange("(kc p) n -> p kc n", p=128)
            nc.sync.dma_start(x_sb, xv[:, :, ds(it, tt)])

            # ---- FiLM from cond (x chunks 4,5) ---------------------------
            fw1_sb = load_w("fw1", 0, 2, 0, FH)
            rw_slabs = preload("rw", KO_X, H, pool=rwpool)
            psf = psum.tile([128, tt], FP32, tag="mm")
            for kc in range(2):
                nc.tensor.matmul(psf, lhsT=fw1_sb[:, kc, :],
                                 rhs=x_sb[:, 4 + kc, :],
                                 start=(kc == 0), stop=(kc == 1))
            # gelu via tanh approximation (abs err <= 3e-4 on fh; attenuated
            # ~15x through the 0.02-scale fw2). bf16 elementwise.
            fh_sb = tmps.tile([128, tt], BF16, tag="fh")
            xx = tmps.tile([128, tt], BF16, tag="gelu_x")
            nc.scalar.activation(xx, psf, AF.Identity, bias=vec("fb1", 0))
            x2 = tmps.tile([128, tt], BF16, tag="gelu_t")
            nc.vector.tensor_mul(x2, xx, xx)
            nc.vector.tensor_mul(x2, x2, xx)  # x^3
            nc.vector.scalar_tensor_tensor(x2, x2, 0.044715, xx,
                                           ALU.mult, ALU.add)
            nc.scalar.activation(x2, x2, AF.Tanh, scale=0.7978845608028654)
            nc.vector.tensor_scalar(x2, x2, 0.5, 0.5, ALU.mult, ALU.add)
            nc.vector.tensor_mul(fh_sb, x2, xx)

            g_sb = acts.tile([128, KO_H, tt], BF16, tag="slotD")  # g/k
            b_sb = acts.tile([128, KO_H, tt], BF16, tag="slotE")  # b/v/m
            for half in range(2):
                w2 = load_w("fw2", 0, 1, half * H, H)
                for mi in range(KO_H):
                    ps = psum.tile([128, tt], FP32, tag="mm")
                    nc.tensor.matmul(ps, lhsT=w2[:, 0, mi * 128 : (mi + 1) * 128],
                                     rhs=fh_sb, start=True, stop=True)
                    tn = tmps.tile([128, tt], BF16, tag="lnt")
                    nc.scalar.activation(tn, ps, AF.Tanh,
                                         bias=vec("fb2", half * 8 + mi))
                    if half == 0:
                        nc.vector.tensor_scalar(g_sb[:, mi, :], tn, 0.5, 1.0,
                                                ALU.mult, ALU.add)
                    else:
                        nc.vector.tensor_scalar_mul(b_sb[:, mi, :], tn, 0.5)

            # ---- encoder layer 1: y1 = x@ew1' + eb1' (zero-mean) --------
            y_sb = acts.tile([128, KO_H, tt], BF16, tag="slotB")  # y1/y2/s
            fm_layer("ew1", KO_X, H, lambda ko: x_sb[:, ko, :],
                     lambda mi, ps: nc.scalar.activation(
                         y_sb[:, mi, :], ps, AF.Identity, bias=vec("eb1", mi)))
            rstd1 = rstd_zeromean(lambda c: y_sb[:, c, :], KO_H)

            # ---- apply LN1 -> h1 = relu(t*g + bt), t = y*rstd -----------
            h1_sb = acts.tile([128, KO_H, tt], BF16, tag="slotF")  # h1/qin
            for c in range(KO_H):
                t = tmps.tile([128, tt], BF16, tag="lnt")
                nc.vector.tensor_mul(t, y_sb[:, c, :], rstd1)
                nc.vector.tensor_scalar(t, t, vec("eg1", c), vec("ebt1", c),
                                        ALU.mult, ALU.add)
                nc.vector.tensor_scalar_max(h1_sb[:, c, :], t, 0.0)

            # ---- encoder layer 2 (zero-mean) + LN2 -> h ------------------
            fm_layer("ew2", KO_H, H, lambda ko: h1_sb[:, ko, :],
                     lambda mi, ps: nc.scalar.activation(
                         y_sb[:, mi, :], ps, AF.Identity, bias=vec("eb2", mi)),
                     slabs_by_m0=ew2_res)
            rstd2 = rstd_zeromean(lambda c: y_sb[:, c, :], KO_H)
            # ---- res = x@rw' + rb' (zero-mean; independent PE work) ------
            res_sb = acts.tile([128, KO_H, tt], BF16, tag="slotC")  # res/h2
            fm_layer("rw", KO_X, H, lambda ko: x_sb[:, ko, :],
                     lambda mi, ps: nc.scalar.activation(
                         res_sb[:, mi, :], ps, AF.Identity, bias=vec("rb", mi)),
                     slabs_by_m0=rw_slabs)

            h_sb = acts.tile([128, KO_H, tt], BF16, tag="slotG")  # h/hr
            for c in range(KO_H):
                t = tmps.tile([128, tt], BF16, tag="lnt")
                nc.vector.tensor_mul(t, y_sb[:, c, :], rstd2)
                nc.vector.tensor_scalar(t, t, vec("eg2", c), vec("ebt2", c),
                                        ALU.mult, ALU.add)
                nc.vector.tensor_scalar_max(h_sb[:, c, :], t, 0.0)

            # ---- LNq stats on h (full: mean + var) ----------------------
            ps_sum = psum.tile([128, tt], FP32, tag="mm")
            for c in range(KO_H):
                nc.tensor.matmul(ps_sum, lhsT=ones, rhs=h_sb[:, c, :],
                                 start=(c == 0), stop=(c == KO_H - 1))
            mean_h = stat.tile([128, tt], FP32, tag="mean")
            nc.scalar.mul(mean_h, ps_sum, 1.0 / float(H))
            meanh_bf = stat.tile([128, tt], BF16, tag="mean_bf")
            nc.vector.tensor_copy(meanh_bf, mean_h)
            m2 = stat.tile([128, tt], FP32, tag="m2")
            nc.vector.tensor_mul(m2, mean_h, mean_h)
            ps_ssq = ssq_psum(lambda c: h_sb[:, c, :], KO_H)
            varq = stat.tile([128, tt], FP32, tag="var")
            nc.vector.scalar_tensor_tensor(varq, ps_ssq, 1.0 / float(H), m2,
                                           ALU.mult, ALU.subtract)
            nc.scalar.activation(varq, varq, AF.Sqrt, bias=eps_sb, scale=1.0)
            rstdq = recip_cast(varq)

            # ---- qin = (h*rstdq - mrsq)*lnq_g*g + (lnq_b*g + b) ---------
            qin_sb = h1_sb  # h1 dead; reuse the slot handle directly
            hmm = acts.tile([128, KO_H, tt], BF16, tag="slotHM")
            for c in range(KO_H):
                # mean-subtract first: independent of rstdq, hides under the
                # sqrt/reciprocal tail
                nc.vector.tensor_sub(hmm[:, c, :], h_sb[:, c, :], meanh_bf)
            for c in range(KO_H):
                t = tmps.tile([128, tt], BF16, tag="lnt")
                nc.vector.tensor_mul(t, hmm[:, c, :], rstdq)
                u = tmps.tile([128, tt], BF16, tag="lnu")
                nc.vector.tensor_scalar(u, t, vec("lnq_g", c), vec("lnq_b", c),
                                        ALU.mult, ALU.add)
                nc.vector.tensor_mul(u, u, g_sb[:, c, :])
                nc.vector.tensor_add(qin_sb[:, c, :], u, b_sb[:, c, :])

            # ---- k = h@wk (emitted first: covers LNq chain on PE) --------
            k_sb = g_sb  # g dead after qin
            fm_layer("wk", KO_H, H, lambda ko: h_sb[:, ko, :],
                     lambda mi, ps: nc.scalar.copy(k_sb[:, mi, :], ps))

            # ---- q = qin@wq ---------------------------------------------
            q_sb = x_sb  # x dead after res/film
            kq = acts.tile([128, 2, tt], BF16, tag="slotA2")  # q chunks 6,7
            def q_out(mi, ps):
                if mi < KO_X:
                    nc.scalar.copy(q_sb[:, mi, :], ps)
                else:
                    nc.scalar.copy(kq[:, mi - KO_X, :], ps)
            fm_layer("wq", KO_H, H, lambda ko: qin_sb[:, ko, :], q_out)

            def q_chunk(hd):
                return q_sb[:, hd, :] if hd < KO_X else kq[:, hd - KO_X, :]

            # ---- v (token-major): lhsT = h chunk, rhs = wv slab ----------
            # wv loaded ONCE per tile (two [128, 8, 512] halves), reused by
            # all four token blocks.
            v_sb = b_sb  # b dead after qin
            wv0 = load_w("wv", 0, 8, 0, 512, pool=wvpool)
            wv1 = load_w("wv", 0, 8, 512, 512, pool=wvpool)
            wv_half = [wv0, wv1]
            for g in range(nblk):
                for half in range(2):
                    slab = wv_half[half]
                    ps = psum.tile([128, tt], FP32, tag="mm")
                    for ko in range(KO_H):
                        nc.tensor.matmul(
                            ps,
                            lhsT=h_sb[:, ko, g * 128 : (g + 1) * 128],
                            rhs=slab[:, ko, :],
                            start=(ko == 0), stop=(ko == KO_H - 1))
                    nc.scalar.copy(v_sb[:, g * 2 + half, :], ps)

            def v_blk(g, hd):
                ch = g * 2 + hd // 4
                return v_sb[:, ch, (hd % 4) * 128 : (hd % 4 + 1) * 128]

            # ---- hrm = (h - mean_h) + res (into h slot; h dead after v) --
            # wo eviction then adds straight onto the pre-mean-subtracted
            # residual stream, so the coupling-LN needs no subtract pass.
            for c in range(KO_H):
                nc.vector.tensor_add(h_sb[:, c, :], hmm[:, c, :],
                                     res_sb[:, c, :])

            # ---- attention per 128-token block (4 batch items) -----------
            s_sb = acts.tile([128, KO_H, tt], BF16, tag="slotS")
            for g in range(nblk):
                ps_s = psatt.tile([128, NH * 128], FP32, tag="att")
                for hd in range(NH):
                    nc.tensor.matmul(
                        ps_s[:, hd * 128 : (hd + 1) * 128],
                        lhsT=k_sb[:, hd, g * 128 : (g + 1) * 128],
                        rhs=q_chunk(hd)[:, g * 128 : (g + 1) * 128],
                        start=True, stop=True)
                exps = attp.tile([128, NH * 128], BF16, tag="exps")
                nc.scalar.activation(exps, ps_s, AF.Exp,
                                     scale=float(1.0 / np.sqrt(DH)))
                ps_d = psatt.tile([128, NH * 128], FP32, tag="att")
                for half in range(2):
                    sl = slice(half * 512, (half + 1) * 512)
                    nc.tensor.matmul(ps_d[:, sl], lhsT=bdt[:, :128],
                                     rhs=exps[:, sl], start=True, stop=True)
                rec = attp.tile([128, NH * 128], BF16, tag="rec", bufs=1)
                nc.vector.reciprocal(rec, ps_d)
                nc.vector.tensor_mul(rec, rec, bdt)
                nc.vector.tensor_mul(exps, exps, rec)
                for hb in range(2):
                    ps_o = psout.tile([128, 512], FP32, tag="opsum")
                    for hh in range(4):
                        hd = hb * 4 + hh
                        nc.tensor.matmul(
                            ps_o[:, hh * 128 : (hh + 1) * 128],
                            lhsT=v_blk(g, hd),
                            rhs=exps[:, hd * 128 : (hd + 1) * 128],
                            start=True, stop=True)
                    nc.scalar.copy(
                        s_sb[:, hb * 4 : (hb + 1) * 4, g * 128 : (g + 1) * 128],
                        ps_o.rearrange("p (h d) -> p h d", h=4))

            # ---- ao = o@wo' ; s = hr + ao (wo' zero-col-mean) ------------
            o_sb = s_sb  # naming: s_sb currently holds o (feature-major)
            s2_sb = res_sb  # res dead (folded into hr); reuse for s
            fm_layer("wo", KO_H, H, lambda ko: o_sb[:, ko, :],
                     lambda mi, ps: nc.vector.tensor_add(
                         s2_sb[:, mi, :], ps, h_sb[:, mi, :]))

            # ---- h2 = LN_cn(s): var_s = ssq_s/H - mean_h^2 --------------
            # s2_sb holds s - mean_s exactly, so var is just ssq/H
            ps_ssq_s = ssq_psum(lambda c: s2_sb[:, c, :], KO_H, on_act=True)
            sdc = stat.tile([128, tt], FP32, tag="sd")
            nc.scalar.activation(sdc, ps_ssq_s, AF.Sqrt, bias=eps_sb,
                                 scale=1.0 / float(H))
            rstdc = recip_cast(sdc)
            h2_sb = h_sb  # hr dead after wo-add
            for c in range(KO_H):
                t = tmps.tile([128, tt], BF16, tag="lnt")
                nc.vector.tensor_mul(t, s2_sb[:, c, :], rstdc)
                nc.vector.tensor_scalar(h2_sb[:, c, :], t, vec("cn_g", c),
                                        vec("cn_b", c), ALU.mult, ALU.add)

            # ---- m = relu(h2@hw1 + hb1) ---------------------------------
            m_sb = v_sb  # v dead after attention
            fm_layer("hw1", KO_H, H, lambda ko: h2_sb[:, ko, :],
                     lambda mi, ps: nc.scalar.activation(
                         m_sb[:, mi, :], ps, AF.Relu, bias=vec("hb1", mi)))

            # ---- out (token-major): lhsT = m chunk, rhs = hw2 -----------
            hw2a = load_w("hw2", 0, 4, 0, OUT)
            hw2b = load_w("hw2", 4, 4, 0, OUT)
            out_sb = outp.tile([128, nblk, OUT], FP32, tag="out", bufs=1)
            for g in range(nblk):
                ps = psum.tile([128, tt], FP32, tag="mm")
                for ko in range(KO_H):
                    slab = hw2a if ko < 4 else hw2b
                    nc.tensor.matmul(
                        ps[:, :OUT],
                        lhsT=m_sb[:, ko, g * 128 : (g + 1) * 128],
                        rhs=slab[:, ko % 4, :],
                        start=(ko == 0), stop=(ko == KO_H - 1))
                nc.vector.tensor_add(out_sb[:, g, :], ps[:, :OUT], hb2bc)
            ov = out_d[ds(it, tt), :].rearrange("(g p) f -> p g f", p=128)
            nc.sync.dma_start(ov, out_sb)

    return nc


# ------------------------------------------------------- walrus wait limit
# (identical to v1 — see kernel.py for rationale)
_ENGINE_PROCS = ("Activation", "DVE", "PE", "Pool", "SP")
_DMA_OPS = ("DMACopy", "DMATranspose", "TriggeredCopy")


def _rewrite_bir_waits(j):
    n_new = 0
    for fn in j.get("functions", []):
        for bb in fn.get("blocks", []):
            out = []
            for inst in bb.get("instructions", []):
                si = inst.get("sync_info")
                waits = (si or {}).get("on_wait") or []
                if len(waits) > 1:
                    eng = inst.get("engine")
                    opc = inst.get("opcode", "")
                    if (eng in _ENGINE_PROCS and opc not in _DMA_OPS
                            and not opc.startswith("DMA")):
                        own = eng + "_"
                        kept = [w for w in waits
                                if not (w.get("ant_name", "").startswith(own)
                                        and w["ant_name"][len(own):].isdigit())]
                        if kept:
                            waits = kept
                    for w in waits[:-1]:
                        out.append({
                            "debug": inst.get("debug"),
                            "engine": inst["engine"],
                            "ins": [], "outs": [],
                            "name": f"WSPLIT-{n_new}",
                            "opcode": "NoOp",
                            "sync_info": {"on_wait": [w], "on_update": []},
                        })
                        n_new += 1
                    si["on_wait"] = [waits[-1]]
                out.append(inst)
            bb["instructions"] = out
    return j, n_new


def _install_wait_splitter():
    import orjson
    import concourse.bass2jax as b2j
    if getattr(b2j, "_wait_split_installed", False):
        return
    orig = b2j.compile_bir_kernel

    def wrapped(bir_json, *args, **kwargs):
        j = orjson.loads(bir_json)
        j, n_new = _rewrite_bir_waits(j)
        return orig(orjson.dumps(j), *args, **kwargs)

    b2j.compile_bir_kernel = wrapped
    b2j._wait_split_installed = True


# ---------------------------------------------------------------- host side
BF16NP = mybir.dt.np(mybir.dt.bfloat16)


def _pack_shared(inputs):
    f32 = lambda a: np.ascontiguousarray(np.asarray(a, dtype=np.float32))
    shared = {}
    # zero-col-mean folding: LN is shift-invariant, so subtracting each
    # weight's per-row output-mean (and the bias mean) leaves the network
    # output unchanged while making pre-LN activations exactly zero-mean.
    ew1 = f32(inputs["ew1"]); ew1 = ew1 - ew1.mean(axis=1, keepdims=True)
    eb1 = f32(inputs["eb1"]); eb1 = eb1 - eb1.mean()
    ew2 = f32(inputs["ew2"]); ew2 = ew2 - ew2.mean(axis=1, keepdims=True)
    eb2 = f32(inputs["eb2"]); eb2 = eb2 - eb2.mean()
    rw = f32(inputs["rw"]); rw = rw - rw.mean(axis=1, keepdims=True)
    rb = f32(inputs["rb"]); rb = rb - rb.mean()
    wo = f32(inputs["wo"]); wo = wo - wo.mean(axis=1, keepdims=True)

    vec_vals = dict(inputs)
    vec_vals["eb1"], vec_vals["eb2"], vec_vals["rb"] = eb1, eb2, rb
    vecs = np.zeros((128, VEC_COLS), dtype=np.float32)
    for name, ncols in VEC_SPECS:
        v = f32(vec_vals[name]).reshape(ncols, 128)
        vecs[:, VEC_OFF[name] : VEC_OFF[name] + ncols] = v.T
    shared["vecs"] = vecs
    shared["hb2bc"] = np.ascontiguousarray(
        np.broadcast_to(f32(inputs["hb2"])[None, :], (128, OUT)))
    bd = np.kron(np.eye(4, dtype=np.float32), np.ones((32, 32), np.float32))
    shared["bdt"] = np.ascontiguousarray(np.tile(bd, (1, NH))).astype(BF16NP)
    folded = {"ew1": ew1, "ew2": ew2, "rw": rw, "wo": wo}
    for name in ("ew1", "ew2", "rw", "fw1", "fw2", "wq", "wk", "wv", "wo",
                 "hw1", "hw2"):
        w = folded.get(name)
        shared[name] = (w if w is not None else f32(inputs[name])).astype(BF16NP)
    return shared


def make_in_maps(inputs, ncores=NCORES, ntok=NTOK):
    shared = _pack_shared(inputs)
    state = np.asarray(inputs["state"], dtype=np.float32)
    cond = np.asarray(inputs["cond"], dtype=np.float32)
    b_loc = state.shape[0] // ncores
    in_maps = []
    for c in range(ncores):
        sl = slice(c * b_loc, (c + 1) * b_loc)
        x = np.concatenate(
            [state[sl].reshape(-1, IN), cond[sl].reshape(-1, CD)], axis=1)
        np.clip(x, -16.0, 16.0, out=x)
        in_maps.append({"x_fm": np.ascontiguousarray(x.T).astype(BF16NP), **shared})
    return in_maps


_CACHE = {}


def _get_program(ntok=NTOK, tt=TT, reps=1, fixed_addr=False):
    key = (ntok, tt, reps, fixed_addr)
    if key not in _CACHE:
        _CACHE[key] = build_program(ntok, tt, reps, fixed_addr)
    return _CACHE[key]


def run(inputs, trace=False):
    """Run on 8 NeuronCores; returns (output [B,T,OUT], BassKernelResults)."""
    _install_wait_splitter()
    nc = _get_program()
    in_maps = make_in_maps(inputs)
    res = run_bass_kernel_spmd(nc, in_maps, list(range(NCORES)), trace=trace)
    outs = [res.results[c]["out_tm"].reshape(B_LOC, T, OUT)
            for c in range(NCORES)]
    return np.concatenate(outs, axis=0), res


def kernel(**inputs) -> np.ndarray:
    out, _ = run(inputs)
    return out


# revision 3
# speedup vs baseline: 1.0024x; 1.0024x over previous
"""
Trainium2 Bass kernel for nn_EventMotionModel (dense transformer block) — v2.

Same math/layout strategy as v1 (feature-major acts, batch-parallel over 8
cores, bf16 matmuls, fp32 psum) with engine-load rebalancing on top:

  * Host-side zero-col-mean folding of ew1/eb1, ew2/eb2, rw/rb, wo.
    LayerNorm is shift-invariant, so subtracting each weight column-mean
    makes the encoder pre-LN activations exactly zero-mean: the mean
    matmul passes, the mean/m2 stat ops, and the mean-subtract in the LN
    applies all disappear for LN1/LN2; the coupling LN reuses mean(h)
    from the query-LN stats (res and o@wo are zero-mean by construction).
  * All elementwise chains run in bf16 (2x/4x DVE modes) with per-LN
    fp32->bf16 casts of the stats; squares for sum-of-squares run on DVE
    as y*y instead of ACT Square.
  * LN applies fuse gamma/beta(+relu) into one ACT op via per-partition
    scale AND bias APs: h = Relu(t*gamma + beta), t = y*rstd.
  * Attention scores: one 128-wide matmul per (block, head) instead of
    four 32-wide ones; softmax normalize/mask muls in bf16; attention
    output evicted with one FD=512 strided ACT op per psum half.
"""

import numpy as np

import concourse.bass as bass
import concourse.tile as tile
from concourse import mybir
from concourse.bass import ds
from concourse.bass_utils import run_bass_kernel_spmd

# ---------------------------------------------------------------- constants
H = 1024
NH = 8
DH = 128
IN = 512
CD = 256
OUT = 512
FH = 128
B, T = 2048, 32
D = IN + CD  # 768

NCORES = 8
B_LOC = B // NCORES          # 256
NTOK = B_LOC * T             # 8192 tokens per core
TT = 512                     # tokens per tile
NBLK = TT // 128             # 128-token blocks per tile (= 4)

FP32 = mybir.dt.float32
BF16 = mybir.dt.bfloat16
AF = mybir.ActivationFunctionType
ALU = mybir.AluOpType

KO_X = D // 128              # 6 feature chunks of x
KO_H = H // 128              # 8 feature chunks of hidden

# packed per-feature vectors: name -> n_cols (=len/128) in the "vecs" input
VEC_SPECS = [
    ("eb1", 8), ("eg1", 8), ("ebt1", 8),
    ("eb2", 8), ("eg2", 8), ("ebt2", 8),
    ("rb", 8),
    ("lnq_g", 8), ("lnq_b", 8),
    ("cn_g", 8), ("cn_b", 8),
    ("hb1", 8),
    ("fb1", 1), ("fb2", 16),
]
VEC_OFF = {}
_off = 0
for _name, _n in VEC_SPECS:
    VEC_OFF[_name] = _off
    _off += _n
VEC_COLS = _off


# ---------------------------------------------------------------- program
def build_program(ntok=NTOK, tt=TT, reps=1, fixed_addr=False):
    import concourse.tile_sem_assignment as _tsa
    _tsa.NUM_HWDGE_SEMS = 4
    nblk = tt // 128
    nc = bass.Bass()

    # DRAM parameters ------------------------------------------------------
    x_fm = nc.declare_dram_parameter("x_fm", [D, ntok + tt], BF16, isOutput=False)
    vecs_d = nc.declare_dram_parameter("vecs", [128, VEC_COLS], FP32, isOutput=False)
    hb2bc_d = nc.declare_dram_parameter("hb2bc", [128, OUT], FP32, isOutput=False)
    bdt_d = nc.declare_dram_parameter("bdt", [128, NH * 128], BF16, isOutput=False)
    w_d = {}
    for name, k, m in [
        ("ew1", D, H), ("ew2", H, H), ("rw", D, H),
        ("fw1", CD, FH), ("fw2", FH, 2 * H),
        ("wq", H, H), ("wk", H, H), ("wv", H, H), ("wo", H, H),
        ("hw1", H, H), ("hw2", H, OUT),
    ]:
        w_d[name] = nc.declare_dram_parameter(name, [k, m], BF16, isOutput=False)
    out_d = nc.declare_dram_parameter("out_tm", [ntok + tt, OUT], FP32, isOutput=True)

    from contextlib import ExitStack

    with tile.TileContext(nc) as tc, ExitStack() as st, \
            nc.allow_low_precision(
                reason="bf16 intermediates feed bf16 matmuls; stats keep "
                       "fp32 until the final per-token scalars"):
        singles = st.enter_context(tc.tile_pool(name="singles", bufs=1))
        acts = st.enter_context(tc.tile_pool(name="acts", bufs=1))
        wpool = st.enter_context(tc.tile_pool(name="wpool", bufs=6))
        wvpool = st.enter_context(tc.tile_pool(name="wvpool", bufs=2))
        rwpool = st.enter_context(tc.tile_pool(name="rwpool", bufs=4))
        fw1pool = st.enter_context(tc.tile_pool(name="fw1pool", bufs=1))
        tmps = st.enter_context(tc.tile_pool(name="tmps", bufs=2))
        stat = st.enter_context(tc.tile_pool(name="stat", bufs=2))
        outp = st.enter_context(tc.tile_pool(name="outp", bufs=2))
        attp = st.enter_context(tc.tile_pool(name="attp", bufs=2))
        psum = st.enter_context(tc.tile_pool(name="psum", bufs=3, space="PSUM"))
        psatt = st.enter_context(tc.tile_pool(name="psatt", bufs=2, space="PSUM"))
        psout = st.enter_context(tc.tile_pool(name="psout", bufs=1, space="PSUM"))

        # resident constants ----------------------------------------------
        vecs = singles.tile([128, VEC_COLS], FP32)
        nc.sync.dma_start(vecs, vecs_d[:, :])
        hb2bc = singles.tile([128, OUT], FP32)
        nc.sync.dma_start(hb2bc, hb2bc_d[:, :])
        bdt = singles.tile([128, NH * 128], BF16)
        nc.sync.dma_start(bdt, bdt_d[:, :])
        ones = singles.tile([128, 128], BF16)
        nc.vector.memset(ones, 1.0)
        eps_sb = singles.tile([128, 1], FP32)
        nc.vector.memset(eps_sb, 1e-5)

        def vec(name, c):
            return vecs[:, VEC_OFF[name] + c : VEC_OFF[name] + c + 1]

        # weight streaming: load a [128, ko_n, m_n] slab of W
        def load_w(name, ko0, ko_n, m0, m_n, pool=None, tag="w"):
            w3 = w_d[name].rearrange("(ko p) m -> p ko m", p=128)
            t = (pool or wpool).tile([128, ko_n, m_n], BF16, tag=tag, name=f"w_{name}_{ko0}_{m0}")
            nc.sync.dma_start(t, w3[:, ko0 : ko0 + ko_n, m0 : m0 + m_n])
            return t

        # dense feature-major layer: act_chunks (list of [128, tt] APs) @ W.
        # consumer(mi, ps) receives each output chunk's psum [128, tt].
        def preload(name, KO, M, pool=None, unique_tags=False):
            kh = (KO + 1) // 2
            out = {}
            for m0 in range(0, M, 512):
                m_n = min(512, M - m0)
                out[m0] = [(k0, load_w(name, k0, min(kh, KO - k0), m0, m_n,
                                       pool=pool,
                                       tag=(f"w_{name}_{m0}_{k0}"
                                            if unique_tags else "w")))
                           for k0 in range(0, KO, kh)]
            return out

        def fm_layer(name, KO, M, act_of, consumer, slabs_by_m0=None,
                     m0_list=None):
            kh = (KO + 1) // 2  # K-halves: 3+3 for 768, 4+4 for 1024
            for m0 in (m0_list if m0_list is not None
                       else range(0, M, 512)):
                m_n = min(512, M - m0)
                if slabs_by_m0 is not None:
                    slabs = slabs_by_m0[m0]
                else:
                    slabs = []
                    for k0 in range(0, KO, kh):
                        k_n = min(kh, KO - k0)
                        slabs.append((k0, load_w(name, k0, k_n, m0, m_n)))
                for mi in range(m_n // 128):
                    ps = psum.tile([128, tt], FP32, tag="mm")
                    for k0, slab in slabs:
                        k_n = slab.shape[1]
                        for kk in range(k_n):
                            ko = k0 + kk
                            nc.tensor.matmul(
                                ps,
                                lhsT=slab[:, kk, mi * 128 : (mi + 1) * 128],
                                rhs=act_of(ko),
                                start=(ko == 0),
                                stop=(ko == KO - 1),
                            )
                    consumer(m0 // 128 + mi, ps)

        # sum of squares over KO chunks of y -> psum [128, tt] (broadcast
        # over partitions).  Squares on DVE in bf16.
        def ssq_psum(y_of, KO, on_act=False):
            ps_ssq = psum.tile([128, tt], FP32, tag="mm")
            for c in range(KO):
                sq = tmps.tile([128, tt], BF16, tag="sq")
                if on_act:
                    nc.scalar.square(sq, y_of(c))
                else:
                    nc.vector.tensor_mul(sq, y_of(c), y_of(c))
                nc.tensor.matmul(ps_ssq, lhsT=ones, rhs=sq,
                                 start=(c == 0), stop=(c == KO - 1))
            return ps_ssq

        # rstd (bf16) for a zero-mean y: rstd = 1/sqrt(ssq/n + eps)
        def recip_cast(sd):
            rstd = stat.tile([128, tt], BF16, tag="rstd_bf")
            nc.vector.reciprocal(rstd, sd)
            return rstd

        def rstd_zeromean(y_of, KO):
            ps_ssq = ssq_psum(y_of, KO)
            sd = stat.tile([128, tt], FP32, tag="sd")
            nc.scalar.activation(sd, ps_ssq, AF.Sqrt, bias=eps_sb,
                                 scale=1.0 / float(KO * 128))
            return recip_cast(sd)

        # carried across iterations: attention output o and the
        # pre-mean-subtracted residual stream hrm of the PREVIOUS tile.
        o_car = singles.tile([128, KO_H, tt], BF16)
        hrm_car = singles.tile([128, KO_H, tt], BF16)
        nc.vector.memset(o_car, 0.0)
        nc.vector.memset(hrm_car, 0.0)

        # ------------------------------------------------- rotated loop
        # Body k = [x/film front of tile k] + [back half (wo..out) of tile
        # k-1, fed from o_car/hrm_car] + [rest of tile k's front].  The
        # back half's ~34us of PE work (wo/hw1/hw2) fills the film/LN-tail
        # stalls of tile k, and tile k's big GEMM block covers the coupling
        # -LN chain.  Output is front-padded one tile (iteration 0's back
        # half computes garbage from the zeroed carries into out rows
        # [0, tt)); the host discards that tile.  One extra iteration
        # drains the last tile's back half.
        with tc.For_i(0, ntok * reps, tt) as it_raw:
            it = 0 if fixed_addr else it_raw
            x_sb = acts.tile([128, KO_X, tt], BF16, tag="slotA")  # x
            xv = x_fm.rearrange("(kc p) n -> p kc n", p=128)
            nc.sync.dma_start(x_sb, xv[:, :, ds(it, tt)])

            # ---- FiLM from cond (x chunks 4,5) ---------------------------
            fw1_sb = load_w("fw1", 0, 2, 0, FH, pool=fw1pool)
            rw_slabs = preload("rw", KO_X, H, pool=rwpool)
            psf = psum.tile([128, tt], FP32, tag="mm")
            for kc in range(2):
                nc.tensor.matmul(psf, lhsT=fw1_sb[:, kc, :],
                                 rhs=x_sb[:, 4 + kc, :],
                                 start=(kc == 0), stop=(kc == 1))
            fh_sb = tmps.tile([128, tt], BF16, tag="fh")
            xx = tmps.tile([128, tt], BF16, tag="gelu_x")
            nc.scalar.activation(xx, psf, AF.Identity, bias=vec("fb1", 0))
            x2 = tmps.tile([128, tt], BF16, tag="gelu_t")
            nc.vector.tensor_mul(x2, xx, xx)
            nc.vector.tensor_mul(x2, x2, xx)  # x^3
            nc.vector.scalar_tensor_tensor(x2, x2, 0.044715, xx,
                                           ALU.mult, ALU.add)
            nc.scalar.activation(x2, x2, AF.Tanh, scale=0.7978845608028654)
            nc.vector.tensor_scalar(x2, x2, 0.5, 0.5, ALU.mult, ALU.add)
            nc.vector.tensor_mul(fh_sb, x2, xx)

            # ======== BACK HALF of tile k-1 (o_car/hrm_car) ==============
            # ---- ao = o@wo' ; s2m = ao + hrm  (wo' zero-col-mean) -------
            s2_sb = acts.tile([128, KO_H, tt], BF16, tag="slotC")  # s2m/res
            fm_layer("wo", KO_H, H, lambda ko: o_car[:, ko, :],
                     lambda mi, ps: nc.vector.tensor_add(
                         s2_sb[:, mi, :], ps, hrm_car[:, mi, :]))

            # ---- h2 = LN_cn(s): s2m is exactly s - mean_s ---------------
            ps_ssq_s = ssq_psum(lambda c: s2_sb[:, c, :], KO_H)
            sdc = stat.tile([128, tt], FP32, tag="sd")
            nc.scalar.activation(sdc, ps_ssq_s, AF.Sqrt, bias=eps_sb,
                                 scale=1.0 / float(H))
            rstdc = recip_cast(sdc)
            h2_sb = acts.tile([128, KO_H, tt], BF16, tag="slotG")  # h2/h
            for c in range(KO_H):
                t = tmps.tile([128, tt], BF16, tag="lnt")
                nc.vector.tensor_mul(t, s2_sb[:, c, :], rstdc)
                nc.vector.tensor_scalar(h2_sb[:, c, :], t, vec("cn_g", c),
                                        vec("cn_b", c), ALU.mult, ALU.add)

            # ---- m = relu(h2@hw1 + hb1) ---------------------------------
            m_sb = acts.tile([128, KO_H, tt], BF16, tag="slotHM")  # m/hmm
            fm_layer("hw1", KO_H, H, lambda ko: h2_sb[:, ko, :],
                     lambda mi, ps: nc.scalar.activation(
                         m_sb[:, mi, :], ps, AF.Relu, bias=vec("hb1", mi)))

            # ---- out (token-major): lhsT = m chunk, rhs = hw2 -----------
            hw2a = load_w("hw2", 0, 4, 0, OUT)
            hw2b = load_w("hw2", 4, 4, 0, OUT)
            out_sb = outp.tile([128, nblk, OUT], FP32, tag="out", bufs=1)
            for g in range(nblk):
                ps = psum.tile([128, tt], FP32, tag="mm")
                for ko in range(KO_H):
                    slab = hw2a if ko < 4 else hw2b
                    nc.tensor.matmul(
                        ps[:, :OUT],
                        lhsT=m_sb[:, ko, g * 128 : (g + 1) * 128],
                        rhs=slab[:, ko % 4, :],
                        start=(ko == 0), stop=(ko == KO_H - 1))
                nc.vector.tensor_add(out_sb[:, g, :], ps[:, :OUT], hb2bc)
            ov = out_d[ds(it, tt), :].rearrange("(g p) f -> p g f", p=128)
            nc.sync.dma_start(ov, out_sb)

            # ======== rest of FRONT of tile k ============================
            g_sb = acts.tile([128, KO_H, tt], BF16, tag="slotD")  # g/k
            b_sb = acts.tile([128, KO_H, tt], BF16, tag="slotE")  # b/v
            for half in range(2):
                w2 = load_w("fw2", 0, 1, half * H, H)
                for mi in range(KO_H):
                    ps = psum.tile([128, tt], FP32, tag="mm")
                    nc.tensor.matmul(ps, lhsT=w2[:, 0, mi * 128 : (mi + 1) * 128],
                                     rhs=fh_sb, start=True, stop=True)
                    tn = tmps.tile([128, tt], BF16, tag="lnt")
                    nc.scalar.activation(tn, ps, AF.Tanh,
                                         bias=vec("fb2", half * 8 + mi))
                    if half == 0:
                        nc.vector.tensor_scalar(g_sb[:, mi, :], tn, 0.5, 1.0,
                                                ALU.mult, ALU.add)
                    else:
                        nc.vector.tensor_scalar_mul(b_sb[:, mi, :], tn, 0.5)

            # ---- encoder layer 1: y1 = x@ew1' + eb1' (zero-mean) --------
            y_sb = acts.tile([128, KO_H, tt], BF16, tag="slotB")  # y1/y2
            fm_layer("ew1", KO_X, H, lambda ko: x_sb[:, ko, :],
                     lambda mi, ps: nc.scalar.activation(
                         y_sb[:, mi, :], ps, AF.Identity, bias=vec("eb1", mi)))
            rstd1 = rstd_zeromean(lambda c: y_sb[:, c, :], KO_H)

            # ---- apply LN1 -> h1 = relu(t*g + bt), t = y*rstd -----------
            h1_sb = acts.tile([128, KO_H, tt], BF16, tag="slotF")  # h1/qin
            for c in range(KO_H):
                t = tmps.tile([128, tt], BF16, tag="lnt")
                nc.vector.tensor_mul(t, y_sb[:, c, :], rstd1)
                nc.vector.tensor_scalar(t, t, vec("eg1", c), vec("ebt1", c),
                                        ALU.mult, ALU.add)
                nc.vector.tensor_scalar_max(h1_sb[:, c, :], t, 0.0)

            # ---- encoder layer 2 (zero-mean) + LN2 -> h ------------------
            fm_layer("ew2", KO_H, H, lambda ko: h1_sb[:, ko, :],
                     lambda mi, ps: nc.scalar.activation(
                         y_sb[:, mi, :], ps, AF.Identity, bias=vec("eb2", mi)))
            rstd2 = rstd_zeromean(lambda c: y_sb[:, c, :], KO_H)
            # ---- res = x@rw' + rb' (zero-mean; fills LN2/LNq windows) ----
            res_sb = acts.tile([128, KO_H, tt], BF16, tag="slotC")  # res
            fm_layer("rw", KO_X, H, lambda ko: x_sb[:, ko, :],
                     lambda mi, ps: nc.scalar.activation(
                         res_sb[:, mi, :], ps, AF.Identity, bias=vec("rb", mi)),
                     slabs_by_m0=rw_slabs)

            h_sb = acts.tile([128, KO_H, tt], BF16, tag="slotG")  # h
            for c in range(KO_H):
                t = tmps.tile([128, tt], BF16, tag="lnt")
                nc.vector.tensor_mul(t, y_sb[:, c, :], rstd2)
                nc.vector.tensor_scalar(t, t, vec("eg2", c), vec("ebt2", c),
                                        ALU.mult, ALU.add)
                nc.vector.tensor_scalar_max(h_sb[:, c, :], t, 0.0)

            # ---- LNq stats on h (full: mean + var) ----------------------
            ps_sum = psum.tile([128, tt], FP32, tag="mm")
            for c in range(KO_H):
                nc.tensor.matmul(ps_sum, lhsT=ones, rhs=h_sb[:, c, :],
                                 start=(c == 0), stop=(c == KO_H - 1))
            mean_h = stat.tile([128, tt], FP32, tag="mean")
            nc.scalar.mul(mean_h, ps_sum, 1.0 / float(H))
            meanh_bf = stat.tile([128, tt], BF16, tag="mean_bf")
            nc.vector.tensor_copy(meanh_bf, mean_h)
            m2 = stat.tile([128, tt], FP32, tag="m2")
            nc.vector.tensor_mul(m2, mean_h, mean_h)
            ps_ssq = ssq_psum(lambda c: h_sb[:, c, :], KO_H)
            varq = stat.tile([128, tt], FP32, tag="var")
            nc.vector.scalar_tensor_tensor(varq, ps_ssq, 1.0 / float(H), m2,
                                           ALU.mult, ALU.subtract)
            nc.scalar.activation(varq, varq, AF.Sqrt, bias=eps_sb, scale=1.0)
            rstdq = recip_cast(varq)

            # ---- qin = ((h-mean)*rstdq*lnq_g + lnq_b)*g + b -------------
            qin_sb = h1_sb  # h1 dead; reuse the slot handle directly
            hmm = acts.tile([128, KO_H, tt], BF16, tag="slotHM")  # hmm
            for c in range(KO_H):
                nc.vector.tensor_sub(hmm[:, c, :], h_sb[:, c, :], meanh_bf)
            for c in range(KO_H):
                t = tmps.tile([128, tt], BF16, tag="lnt")
                nc.vector.tensor_mul(t, hmm[:, c, :], rstdq)
                u = tmps.tile([128, tt], BF16, tag="lnu")
                nc.vector.tensor_scalar(u, t, vec("lnq_g", c), vec("lnq_b", c),
                                        ALU.mult, ALU.add)
                nc.vector.tensor_mul(u, u, g_sb[:, c, :])
                nc.vector.tensor_add(qin_sb[:, c, :], u, b_sb[:, c, :])

            # ---- k = h@wk -----------------------------------------------
            k_sb = g_sb  # g dead after qin
            fm_layer("wk", KO_H, H, lambda ko: h_sb[:, ko, :],
                     lambda mi, ps: nc.scalar.copy(k_sb[:, mi, :], ps))

            # ---- q = qin@wq ---------------------------------------------
            q_sb = x_sb  # x dead after res/film
            kq = acts.tile([128, 2, tt], BF16, tag="slotA2")  # q chunks 6,7
            def q_out(mi, ps):
                if mi < KO_X:
                    nc.scalar.copy(q_sb[:, mi, :], ps)
                else:
                    nc.scalar.copy(kq[:, mi - KO_X, :], ps)
            fm_layer("wq", KO_H, H, lambda ko: qin_sb[:, ko, :], q_out)

            def q_chunk(hd):
                return q_sb[:, hd, :] if hd < KO_X else kq[:, hd - KO_X, :]

            # ---- v (token-major): wv loaded once, all four blocks -------
            v_sb = b_sb  # b dead after qin
            wv0 = load_w("wv", 0, 8, 0, 512, pool=wvpool)
            wv1 = load_w("wv", 0, 8, 512, 512, pool=wvpool)
            wv_half = [wv0, wv1]
            for g in range(nblk):
                for half in range(2):
                    slab = wv_half[half]
                    ps = psum.tile([128, tt], FP32, tag="mm")
                    for ko in range(KO_H):
                        nc.tensor.matmul(
                            ps,
                            lhsT=h_sb[:, ko, g * 128 : (g + 1) * 128],
                            rhs=slab[:, ko, :],
                            start=(ko == 0), stop=(ko == KO_H - 1))
                    nc.scalar.copy(v_sb[:, g * 2 + half, :], ps)

            def v_blk(g, hd):
                ch = g * 2 + hd // 4
                return v_sb[:, ch, (hd % 4) * 128 : (hd % 4 + 1) * 128]

            # ---- hrm = (h - mean_h) + res -> carried ---------------------
            for c in range(KO_H):
                nc.vector.tensor_add(hrm_car[:, c, :], hmm[:, c, :],
                                     res_sb[:, c, :])

            # ---- attention -> o_car (carried) ----------------------------
            for g in range(nblk):
                ps_s = psatt.tile([128, NH * 128], FP32, tag="att")
                for hd in range(NH):
                    nc.tensor.matmul(
                        ps_s[:, hd * 128 : (hd + 1) * 128],
                        lhsT=k_sb[:, hd, g * 128 : (g + 1) * 128],
                        rhs=q_chunk(hd)[:, g * 128 : (g + 1) * 128],
                        start=True, stop=True)
                exps = attp.tile([128, NH * 128], BF16, tag="exps")
                nc.scalar.activation(exps, ps_s, AF.Exp,
                                     scale=float(1.0 / np.sqrt(DH)))
                ps_d = psatt.tile([128, NH * 128], FP32, tag="att")
                for half in range(2):
                    sl = slice(half * 512, (half + 1) * 512)
                    nc.tensor.matmul(ps_d[:, sl], lhsT=bdt[:, :128],
                                     rhs=exps[:, sl], start=True, stop=True)
                rec = attp.tile([128, NH * 128], BF16, tag="rec", bufs=1)
                nc.vector.reciprocal(rec, ps_d)
                nc.vector.tensor_mul(rec, rec, bdt)
                nc.vector.tensor_mul(exps, exps, rec)
                for hb in range(2):
                    ps_o = psout.tile([128, 512], FP32, tag="opsum")
                    for hh in range(4):
                        hd = hb * 4 + hh
                        nc.tensor.matmul(
                            ps_o[:, hh * 128 : (hh + 1) * 128],
                            lhsT=v_blk(g, hd),
                            rhs=exps[:, hd * 128 : (hd + 1) * 128],
                            start=True, stop=True)
                    nc.scalar.copy(
                        o_car[:, hb * 4 : (hb + 1) * 4, g * 128 : (g + 1) * 128],
                        ps_o.rearrange("p (h d) -> p h d", h=4))


        # ---------------- peeled drain: back half of the final tile ------
        # The loop body pairs back(k-1) with front(k); the last front's
        # o_car/hrm_car are consumed here, writing the final tile at the
        # constant row offset ntok (host reads rows [tt, ntok+tt)).
        it_last = 0 if fixed_addr else ntok * reps
        wo_slabs_d = preload("wo", KO_H, H)
        hw2a_d = load_w("hw2", 0, 4, 0, OUT)
        hw2b_d = load_w("hw2", 4, 4, 0, OUT)
        s2_d = acts.tile([128, KO_H, tt], BF16, tag="slotC")
        fm_layer("wo", KO_H, H, lambda ko: o_car[:, ko, :],
                 lambda mi, ps: nc.vector.tensor_add(
                     s2_d[:, mi, :], ps, hrm_car[:, mi, :]),
                 slabs_by_m0=wo_slabs_d)
        ps_ssq_d = ssq_psum(lambda c: s2_d[:, c, :], KO_H)
        sd_d = stat.tile([128, tt], FP32, tag="sd")
        nc.scalar.activation(sd_d, ps_ssq_d, AF.Sqrt, bias=eps_sb,
                             scale=1.0 / float(H))
        rstd_d = recip_cast(sd_d)
        h2_d = acts.tile([128, KO_H, tt], BF16, tag="slotG")
        for c in range(KO_H):
            t = tmps.tile([128, tt], BF16, tag="lnt")
            nc.vector.tensor_mul(t, s2_d[:, c, :], rstd_d)
            nc.vector.tensor_scalar(h2_d[:, c, :], t, vec("cn_g", c),
                                    vec("cn_b", c), ALU.mult, ALU.add)
        m_d = acts.tile([128, KO_H, tt], BF16, tag="slotHM")
        fm_layer("hw1", KO_H, H, lambda ko: h2_d[:, ko, :],
                 lambda mi, ps: nc.scalar.activation(
                     m_d[:, mi, :], ps, AF.Relu, bias=vec("hb1", mi)))
        out_sb_d = outp.tile([128, nblk, OUT], FP32, tag="out", bufs=1)
        for g in range(nblk):
            ps = psum.tile([128, tt], FP32, tag="mm")
            for ko in range(KO_H):
                slab = hw2a_d if ko < 4 else hw2b_d
                nc.tensor.matmul(
                    ps[:, :OUT],
                    lhsT=m_d[:, ko, g * 128 : (g + 1) * 128],
                    rhs=slab[:, ko % 4, :],
                    start=(ko == 0), stop=(ko == KO_H - 1))
            nc.vector.tensor_add(out_sb_d[:, g, :], ps[:, :OUT], hb2bc)
        ov_d = out_d[ds(it_last, tt), :].rearrange("(g p) f -> p g f", p=128)
        nc.sync.dma_start(ov_d, out_sb_d)

    return nc


# ------------------------------------------------------- walrus wait limit
# (identical to v1 — see kernel.py for rationale)
_ENGINE_PROCS = ("Activation", "DVE", "PE", "Pool", "SP")
_DMA_OPS = ("DMACopy", "DMATranspose", "TriggeredCopy")


def _rewrite_bir_waits(j):
    n_new = 0
    for fn in j.get("functions", []):
        for bb in fn.get("blocks", []):
            out = []
            for inst in bb.get("instructions", []):
                si = inst.get("sync_info")
                waits = (si or {}).get("on_wait") or []
                if len(waits) > 1:
                    eng = inst.get("engine")
                    opc = inst.get("opcode", "")
                    if (eng in _ENGINE_PROCS and opc not in _DMA_OPS
                            and not opc.startswith("DMA")):
                        own = eng + "_"
                        kept = [w for w in waits
                                if not (w.get("ant_name", "").startswith(own)
                                        and w["ant_name"][len(own):].isdigit())]
                        if kept:
                            waits = kept
                    for w in waits[:-1]:
                        out.append({
                            "debug": inst.get("debug"),
                            "engine": inst["engine"],
                            "ins": [], "outs": [],
                            "name": f"WSPLIT-{n_new}",
                            "opcode": "NoOp",
                            "sync_info": {"on_wait": [w], "on_update": []},
                        })
                        n_new += 1
                    si["on_wait"] = [waits[-1]]
                out.append(inst)
            bb["instructions"] = out
    return j, n_new


def _install_wait_splitter():
    import orjson
    import concourse.bass2jax as b2j
    if getattr(b2j, "_wait_split_installed", False):
        return
    orig = b2j.compile_bir_kernel

    def wrapped(bir_json, *args, **kwargs):
        j = orjson.loads(bir_json)
        j, n_new = _rewrite_bir_waits(j)
        return orig(orjson.dumps(j), *args, **kwargs)

    b2j.compile_bir_kernel = wrapped
    b2j._wait_split_installed = True


# ---------------------------------------------------------------- host side
BF16NP = mybir.dt.np(mybir.dt.bfloat16)


def _pack_shared(inputs):
    f32 = lambda a: np.ascontiguousarray(np.asarray(a, dtype=np.float32))
    shared = {}
    # zero-col-mean folding: LN is shift-invariant, so subtracting each
    # weight's per-row output-mean (and the bias mean) leaves the network
    # output unchanged while making pre-LN activations exactly zero-mean.
    ew1 = f32(inputs["ew1"]); ew1 = ew1 - ew1.mean(axis=1, keepdims=True)
    eb1 = f32(inputs["eb1"]); eb1 = eb1 - eb1.mean()
    ew2 = f32(inputs["ew2"]); ew2 = ew2 - ew2.mean(axis=1, keepdims=True)
    eb2 = f32(inputs["eb2"]); eb2 = eb2 - eb2.mean()
    rw = f32(inputs["rw"]); rw = rw - rw.mean(axis=1, keepdims=True)
    rb = f32(inputs["rb"]); rb = rb - rb.mean()
    wo = f32(inputs["wo"]); wo = wo - wo.mean(axis=1, keepdims=True)

    vec_vals = dict(inputs)
    vec_vals["eb1"], vec_vals["eb2"], vec_vals["rb"] = eb1, eb2, rb
    vecs = np.zeros((128, VEC_COLS), dtype=np.float32)
    for name, ncols in VEC_SPECS:
        v = f32(vec_vals[name]).reshape(ncols, 128)
        vecs[:, VEC_OFF[name] : VEC_OFF[name] + ncols] = v.T
    shared["vecs"] = vecs
    shared["hb2bc"] = np.ascontiguousarray(
        np.broadcast_to(f32(inputs["hb2"])[None, :], (128, OUT)))
    bd = np.kron(np.eye(4, dtype=np.float32), np.ones((32, 32), np.float32))
    shared["bdt"] = np.ascontiguousarray(np.tile(bd, (1, NH))).astype(BF16NP)
    folded = {"ew1": ew1, "ew2": ew2, "rw": rw, "wo": wo}
    for name in ("ew1", "ew2", "rw", "fw1", "fw2", "wq", "wk", "wv", "wo",
                 "hw1", "hw2"):
        w = folded.get(name)
        shared[name] = (w if w is not None else f32(inputs[name])).astype(BF16NP)
    return shared


def make_in_maps(inputs, ncores=NCORES, ntok=NTOK):
    shared = _pack_shared(inputs)
    state = np.asarray(inputs["state"], dtype=np.float32)
    cond = np.asarray(inputs["cond"], dtype=np.float32)
    b_loc = state.shape[0] // ncores
    in_maps = []
    for c in range(ncores):
        sl = slice(c * b_loc, (c + 1) * b_loc)
        x = np.concatenate(
            [state[sl].reshape(-1, IN), cond[sl].reshape(-1, CD)], axis=1)
        np.clip(x, -16.0, 16.0, out=x)
        xt = np.ascontiguousarray(x.T).astype(BF16NP)
        # one tile of zero padding: the rotated loop's final iteration
        # reads (and discards) one tile past the real tokens
        xt = np.concatenate(
            [xt, np.zeros((D, TT), dtype=BF16NP)], axis=1)
        in_maps.append({"x_fm": np.ascontiguousarray(xt), **shared})
    return in_maps


_CACHE = {}


def _get_program(ntok=NTOK, tt=TT, reps=1, fixed_addr=False):
    key = (ntok, tt, reps, fixed_addr)
    if key not in _CACHE:
        _CACHE[key] = build_program(ntok, tt, reps, fixed_addr)
    return _CACHE[key]


def run(inputs, trace=False):
    """Run on 8 NeuronCores; returns (output [B,T,OUT], BassKernelResults)."""
    _install_wait_splitter()
    nc = _get_program()
    in_maps = make_in_maps(inputs)
    res = run_bass_kernel_spmd(nc, in_maps, list(range(NCORES)), trace=trace)
    # rows [0, TT) hold iteration 0's garbage back-half output
    outs = [res.results[c]["out_tm"][TT:].reshape(B_LOC, T, OUT)
            for c in range(NCORES)]
    return np.concatenate(outs, axis=0), res


def kernel(**inputs) -> np.ndarray:
    out, _ = run(inputs)
    return out
